# revision 29
# baseline (speedup 1.0000x reference)
"""Trainium2 Bass kernel for nn_CeptaContextBlock (B=4, T=4096, D=1024, P=512, ALPHA=4, PR=64).

Math (after algebraic simplification of the reference):
    W_comb = W_toP + sum_a W_U[:,:,a] * W_V[:,a]          (host precompute)
    WB     = W_comb @ B_mat                               (host precompute)
    Wcw    = W_comb @ W_fromP                             (host precompute)
    MCW    = C_mat @ W_fromP                              (host precompute)
    Fg   = sigmoid(x @ W_F)                               (B,T,P)
    lam  = sigmoid(Fg @ W_lam)                            (B,T,PR)
    u    = x @ WB                                         (B,T,PR)
    s    = scan: s_i = lam_i * s_{i-1} + u_i along T      (B,T,PR)
    h    = x @ Wcw + s @ MCW                              (B,T,D)

The t = x@W_comb intermediate of the reference is folded away: h only needs
x (weights folded on host) plus the tiny rank-64 s contribution, so the big
output matmul stream depends on nothing but the x DMA and the per-chunk scan.

Sharding: 8 cores; core c handles batch b=c//2, token half c%2 (2048 tokens).
The scan carry across each (even, odd) core pair is exchanged with a tiny
AllGather of the final local scan state.

Precision: the gate path (Fg logits, lam logits) runs in fp8 DoubleRow at 2x
PE rate - the sigmoids squash the quantization error and the scan barely
feels it (verified 8.7e-3 end-to-end vs the 2e-2 gate). The u and h paths
stay bf16.

DMA: inputs are single-writer tiles (multi-writer tiles get serialized by
the Tile dependency tracker), issued on the SP and GpSimd queues in strict
alternation of ~0.5MB items in global need order, so the two queues drain
in step and arrival order == need order at the ~325GB/s aggregate HBM rate.
The Activation queue carries no input doorbells so the sigmoid table load
and activations are never stuck behind transfers. wf8 is 4 per-m tiles and
x8-chunk0 is 4 per-kpair tiles so the first matmul needs only 256KB of DMA.

Phases:
  A (scan-critical): Fg fp8-DR, u bf16, lam fp8-DR, chained tensor_tensor_scan
    per 512-token chunk, software-pipelined so each chunk's lam matmuls run
    under cover of the next chunk's Fg matmuls. Phase A is DMA-bound: its
    6.65MB of inputs at ~325GB/s roughly match its ~23us of PE work. The
    carry leaves for the collective ~45us in.
  B (h-stream): for each 128-token tile, accumulate 8 k-matmuls of x@Wcw
    plus one rank-64 s@MCW matmul into the same PSUM group, vector-copy to
    SBUF bf16, DMA out. Token tile 0 runs LAST: its PSUM group additionally
    accumulates the low-rank carry fix cumprod(lam)*carry @ MCW, which by
    then has long cleared the collective. cumprod(lam) underflows within
    ~128 tokens so only tile 0 needs fixing. Even cores mask the carry to
    zero via a per-core {0,1} input (branch-free SPMD).
"""

import os
import sys

import numpy as np

for _p in ("/opt/trn_rl_repo", "/root/.axon_site/_ro/trn_rl_repo"):
    if os.path.isdir(_p) and _p not in sys.path:
        sys.path.append(_p)

import ml_dtypes

import concourse.bass as bass
import concourse.bacc as bacc
import concourse.mybir as mybir
import concourse.tile as tile
from concourse.tile_rust import add_dep_helper
from concourse import bass_utils

B, T, D, P, ALPHA, PR = 4, 4096, 1024, 512, 4, 64
NCORES = 8
TL = T // 2          # tokens per core
KD = D // 128        # 8 d-chunks (contraction for the big matmuls)
KP = KD // 2         # 4 k-pairs for fp8 DoubleRow
PT = P // 128        # 4 p-tiles
CH = 512             # token chunk (free dim per matmul)
NCH = TL // CH       # 4 token chunks per core
CT = 128             # carry-corrected tokens (= token tile 0)
NTT = TL // 128      # 16 token tiles in the h stream
F8S = 32.0           # fp8 W_F pre-scale (undone in the sigmoid activation)
F8L = 64.0           # fp8 W_lam pre-scale
F32 = mybir.dt.float32
BF16 = mybir.dt.bfloat16
FP8 = mybir.dt.float8e4
SIG = mybir.ActivationFunctionType.Sigmoid
CPY = mybir.ActivationFunctionType.Copy
MUL = mybir.AluOpType.mult
ADD = mybir.AluOpType.add
BYP = mybir.AluOpType.bypass
DR = mybir.MatmulPerfMode.DoubleRow

_CACHE = {}


def build_program(ncores: int = NCORES):
    """Build the SPMD Tile program (same NEFF on all cores)."""
    nc = bacc.Bacc(
        "TRN2", target_bir_lowering=False, debug=False, num_devices=ncores
    )

    # big inputs are pre-swizzled on the host to partition-major layout so
    # every DMA lands as 128 fully-contiguous per-partition runs
    x8_d = nc.dram_tensor("x8", [128, NCH * KD * CH], FP8, kind="ExternalInput")
    xb_d = nc.dram_tensor("xb", [128, NCH * KD * CH], BF16, kind="ExternalInput")
    wf8_d = nc.dram_tensor("wf8", [128, KD * P], FP8, kind="ExternalInput")
    wb_d = nc.dram_tensor("wb", [128, KD * PR], BF16, kind="ExternalInput")
    wlam_d = nc.dram_tensor("wlam", [128, PT * PR], FP8, kind="ExternalInput")
    wcw_d = nc.dram_tensor("wcw", [128, KD * D], BF16, kind="ExternalInput")
    mcw_d = nc.dram_tensor("mcw", [PR, D], BF16, kind="ExternalInput")
    cmask_d = nc.dram_tensor("cmask", [PR, 1], F32, kind="ExternalInput")
    h_d = nc.dram_tensor("h", [TL, D], BF16, kind="ExternalOutput")

    with tile.TileContext(nc) as tc:
        with (
            tc.tile_pool(name="wp", bufs=1) as wp,
            tc.tile_pool(name="xp", bufs=8) as xp,
            tc.tile_pool(name="x8p", bufs=4) as x8p,
            tc.tile_pool(name="big", bufs=1) as big,
            tc.tile_pool(name="hp", bufs=6) as hp,
            tc.tile_pool(name="ppa", bufs=2, space="PSUM") as ppa,
            tc.tile_pool(name="pps", bufs=2, space="PSUM") as pps,
            tc.tile_pool(name="pph", bufs=4, space="PSUM") as pph,
            tc.tile_pool(name="dram", bufs=1, space="DRAM") as dp,
        ):
            # ---- input DMAs: single-writer tiles, round-robin across the
            # SP and GpSimd queues in global need order (so arrival order ==
            # need order); the Activation queue stays doorbell-free so the
            # sigmoid table load isn't stuck behind transfers ----
            XC = KD * CH
            HXC = XC // 2
            # wf8 as 4 per-m tiles and x8-chunk0 as 4 per-kpair tiles: the
            # first matmul then needs only 256KB of DMA, and Fg(c0)
            # pipelines with the transfers instead of waiting for 1MB
            wf8_m = [
                wp.tile([128, KD * 128], FP8, tag=f"wf8{m}", name=f"wf8_{m}")
                for m in range(PT)
            ]
            x80_kp = [
                x8p.tile([128, 2 * CH], FP8, tag=f"x80{j}", name=f"x80_{j}")
                for j in range(KP)
            ]
            x8_tiles = [None] + [
                x8p.tile([128, XC], FP8, tag="x8", name=f"x8_{c}")
                for c in range(1, NCH)
            ]
            # xb chunk split into k-halves (separate tiles) for queue balance
            xb_tiles = [
                [
                    xp.tile([128, HXC], BF16, tag="xt", name=f"xb{c}_{hh}")
                    for hh in range(2)
                ]
                for c in range(NCH)
            ]
            wb_sb = wp.tile([128, KD * PR], BF16, tag="wb", name="wb_sb")
            wlam_sb = wp.tile([128, PT * PR], FP8, tag="wlam", name="wlam_sb")
            wcw_tiles = [
                wp.tile([128, KD * D // 4], BF16, tag=f"wcw{q}", name=f"wcw{q}")
                for q in range(4)
            ]
            mcw_sb = wp.tile([PR, D], BF16, tag="mcw", name="mcw_sb")
            cmask_sb = wp.tile([PR, 1], F32, tag="cmask", name="cmask_sb")

            S, G = nc.sync, nc.gpsimd
            for j in range(KP):
                S.dma_start(
                    x80_kp[j][:], x8_d[:, 2 * j * CH : (2 * j + 2) * CH]
                )
                G.dma_start(wf8_m[j][:], wf8_d[:, j * KD * 128 : (j + 1) * KD * 128])
            # strict S/G alternation of ~0.5MB items in global need order so
            # both queues drain in step and arrival order == need order
            S.dma_start(xb_tiles[0][0][:], xb_d[:, 0:HXC])
            G.dma_start(xb_tiles[0][1][:], xb_d[:, HXC:XC])
            S.dma_start(wb_sb[:], wb_d[:, :])
            G.dma_start(wlam_sb[:], wlam_d[:, :])
            seq = (S, G)
            for c in range(1, NCH):
                qi = c % 2
                seq[qi].dma_start(
                    x8_tiles[c][:], x8_d[:, c * XC : (c + 1) * XC]
                )
                seq[1 - qi].dma_start(
                    xb_tiles[c][0][:], xb_d[:, c * XC : c * XC + HXC]
                )
                seq[qi].dma_start(
                    xb_tiles[c][1][:], xb_d[:, c * XC + HXC : (c + 1) * XC]
                )
            QW = KD * D // 4
            for q in range(4):
                seq[q % 2].dma_start(
                    wcw_tiles[q][:], wcw_d[:, q * QW : (q + 1) * QW]
                )
            G.dma_start(mcw_sb[:], mcw_d[:, :])
            S.dma_start(cmask_sb[:], cmask_d[:, :])

            wf8_mv = [t[:].rearrange("p (k q) -> p k q", k=KD) for t in wf8_m]
            wb_v = wb_sb[:].rearrange("p (k q) -> p k q", k=KD)
            wlam_v = wlam_sb[:].rearrange("p (m q) -> p m q", m=PT)
            wcw_vh = [
                t[:].rearrange("p (k q) -> p k q", k=2) for t in wcw_tiles
            ]
            x80_v = [t[:].rearrange("p (k t) -> p k t", k=2) for t in x80_kp]
            x8_v = [None] + [
                t[:].rearrange("p (k t) -> p k t", k=KD) for t in x8_tiles[1:]
            ]
            xb_vh = [
                [t[:].rearrange("p (k t) -> p k t", k=KD // 2) for t in pair]
                for pair in xb_tiles
            ]

            # ---- persistent activations ----
            fg_sb = big.tile([128, PT * TL], FP8, tag="fg", name="fg")
            fg_v = fg_sb[:].rearrange("p (m t) -> p m t", m=PT)
            lam_sb = big.tile([PR, TL], F32, tag="lam", name="lam")
            s1_sb = big.tile([PR, TL], F32, tag="s1", name="s1")
            sloc_sb = big.tile([PR, TL], BF16, tag="sloc", name="sloc")
            cp_sb = big.tile([PR, CT], F32, tag="cp", name="cp")
            cpc_sb = big.tile([PR, CT], BF16, tag="cpc", name="cpc")
            ceff_sb = big.tile([PR, 1], F32, tag="ceff", name="ceff")
            carry_sb = big.tile([PR, 1], F32, tag="carry", name="carry")

            # ---- phase A blocks (scan-critical: Fg fp8-DR, u, lam fp8-DR) ----
            def fg_part(c, ms):
                cs = slice(c * CH, (c + 1) * CH)
                for m in ms:  # Fg, fp8 DoubleRow over 4 k-pairs
                    pa = ppa.tile([128, CH], F32, tag="pa", name=f"pa{c}_{m}")
                    for j in range(KP):
                        x8mov = (
                            x80_v[j][:, 0:2, :]
                            if c == 0
                            else x8_v[c][:, 2 * j : 2 * j + 2, :]
                        )
                        nc.tensor.matmul(
                            pa[:],
                            wf8_mv[m][:, 2 * j : 2 * j + 2, :],
                            x8mov,
                            start=(j == 0),
                            stop=(j == KP - 1),
                            perf_mode=DR,
                        )
                    nc.scalar.activation(
                        fg_v[:, m, cs], pa[:], SIG, scale=1.0 / F8S
                    )

            def ulam_block(c):
                cs = slice(c * CH, (c + 1) * CH)
                # u = x @ WB; placed before lam so its matmuls cover the last
                # Fg sigmoid's latency
                pu = pps.tile([PR, CH], F32, tag="ps", name=f"pu{c}")
                for k in range(KD):
                    nc.tensor.matmul(
                        pu[:],
                        wb_v[:, k, :],
                        xb_vh[c][k // 4][:, k % 4, :],
                        start=(k == 0),
                        stop=(k == KD - 1),
                    )
                # lam = sigmoid(Fg @ W_lam), fp8 DoubleRow over 2 m-pairs
                pl = pps.tile([PR, CH], F32, tag="ps", name=f"pl{c}")
                for j in range(PT // 2):
                    nc.tensor.matmul(
                        pl[:],
                        wlam_v[:, 2 * j : 2 * j + 2, :],
                        fg_v[:, 2 * j : 2 * j + 2, cs],
                        start=(j == 0),
                        stop=(j == PT // 2 - 1),
                        perf_mode=DR,
                    )
                nc.scalar.activation(lam_sb[:, cs], pl[:], SIG, scale=1.0 / F8L)
                # chained local scan; u consumed straight from PSUM
                init = 0.0 if c == 0 else s1_sb[:, c * CH - 1 : c * CH]
                nc.vector.tensor_tensor_scan(
                    s1_sb[:, cs], lam_sb[:, cs], pu[:], init, op0=MUL, op1=ADD
                )
                if c == 0:
                    nc.vector.tensor_tensor_scan(
                        cp_sb[:], lam_sb[:, 0:CT], lam_sb[:, 0:CT], 1.0,
                        op0=MUL, op1=BYP,
                    )
                nc.vector.tensor_copy(sloc_sb[:, cs], s1_sb[:, cs])

            # ---- h stream tile (x@Wcw + s@MCW, carry fix on tile 0) ----
            def h_tile(tt, corr=False, anchor=None, drain_split=1):
                c, tloc = tt // 4, tt % 4
                ts_ = slice(tt * 128, (tt + 1) * 128)
                tls = slice(tloc * 128, (tloc + 1) * 128)
                phs = [
                    pph.tile([128, CH], F32, tag="ph", name=f"ph{tt}_{dc}")
                    for dc in range(2)
                ]
                last = None
                for k in range(KD):
                    for dc in range(2):
                        last = nc.tensor.matmul(
                            phs[dc][:],
                            xb_vh[c][k // 4][:, k % 4, tls],
                            wcw_vh[k // 2][:, k % 2, dc * CH : (dc + 1) * CH],
                            start=(k == 0),
                            stop=False,
                        )
                for dc in range(2):
                    last = nc.tensor.matmul(
                        phs[dc][:],
                        sloc_sb[:, ts_],
                        mcw_sb[:, dc * CH : (dc + 1) * CH],
                        start=False,
                        stop=not corr,
                    )
                if corr:
                    for dc in range(2):
                        cmm = nc.tensor.matmul(
                            phs[dc][:],
                            cpc_sb[:],
                            mcw_sb[:, dc * CH : (dc + 1) * CH],
                            start=False,
                            stop=True,
                        )
                        if anchor is not None:
                            add_dep_helper(
                                cmm.ins, anchor.ins, sync=False,
                                reason="carry fix runs after the h stream",
                            )
                for dc in range(2):
                    ht = hp.tile([128, CH], BF16, tag="hs", name=f"h{tt}_{dc}")
                    dw = CH // drain_split
                    for q in range(drain_split):
                        qs = slice(q * dw, (q + 1) * dw)
                        nc.vector.tensor_copy(ht[:, qs], phs[dc][:, qs])
                        nc.sync.dma_start(
                            h_d[ts_, dc * CH + q * dw : dc * CH + (q + 1) * dw],
                            ht[:, qs],
                        )
                return last

            # ---- schedule: phase A software-pipelined so each chunk's lam
            # matmuls run two Fg m-groups later - the sigmoid chain of chunk
            # c-1 completes under cover of chunk c's matmuls ----
            fg_part(0, range(PT))
            for c in range(1, NCH):
                fg_part(c, (0, 1))
                ulam_block(c - 1)
                fg_part(c, (2, 3))
            ulam_block(NCH - 1)

            # ---- carry exchange (fires while the h stream fills the PE) ----
            cin_bounce = dp.tile([PR, 1], F32, name="cin_bounce")
            cout_bounce = dp.tile([2 * PR, 1], F32, name="cout_bounce")
            nc.gpsimd.dma_start(cin_bounce[:], s1_sb[:, TL - 1 : TL])
            nc.gpsimd.collective_compute(
                "AllGather",
                BYP,
                replica_groups=[[0, 1], [2, 3], [4, 5], [6, 7]],
                ins=[cin_bounce.opt()],
                outs=[cout_bounce.opt()],
            )
            nc.gpsimd.dma_start(carry_sb[:], cout_bounce[0:PR, :])
            # on the Scalar engine: the Vector queue must not head-of-line
            # block on the collective (it carries the h PSUM->SBUF copies)
            nc.scalar.mul(ceff_sb[:], carry_sb[:], cmask_sb[:])
            nc.scalar.mul(cpc_sb[:], cp_sb[:], ceff_sb[:])

            anchor = None
            for tt in range(1, NTT):
                mm = h_tile(tt)
                if tt == 10:
                    anchor = mm
            h_tile(0, corr=True, anchor=anchor, drain_split=2)

    nc.compile()
    return nc


def _prep_inputs(x, W_toP, W_U, W_F, W_V, W_lam, B_mat, C_mat, W_fromP):
    """Host-side sharding prep: weight folds, bf16/fp8 casts, per-core x swizzle."""
    bf = ml_dtypes.bfloat16
    f8 = ml_dtypes.float8_e4m3

    def swz(w):
        # [K*128, q] -> partition-major [128, K*q]
        kq = w.shape[0] // 128
        return np.ascontiguousarray(
            w.reshape(kq, 128, w.shape[1]).transpose(1, 0, 2).reshape(128, -1)
        )

    W_comb = (W_toP + (np.asarray(W_U, np.float32) * W_V[None, :, :]).sum(-1)).astype(
        np.float32
    )
    WB = W_comb @ np.asarray(B_mat, np.float32)
    Wcw = W_comb @ np.asarray(W_fromP, np.float32)
    MCW = np.asarray(C_mat, np.float32) @ np.asarray(W_fromP, np.float32)
    # wf8 host layout: m-major blocks of (k, 128) so each per-m tile's DMA
    # is a contiguous column range
    wf8 = np.concatenate(
        [
            swz(np.clip(np.asarray(W_F[:, m * 128 : (m + 1) * 128], np.float32) * F8S, -240, 240))
            for m in range(PT)
        ],
        axis=1,
    ).astype(f8)
    wb = swz(WB).astype(bf)
    wlam = swz(np.clip(np.asarray(W_lam, np.float32) * F8L, -240, 240)).astype(f8)
    wcw = swz(Wcw).astype(bf)
    mcw = MCW.astype(bf)
    in_maps = []
    for c in range(NCORES):
        b, half = c // 2, c % 2
        xT = np.asarray(x[b, half * TL : (half + 1) * TL, :], np.float32).T
        # [D, TL] -> [128, NCH*KD*CH] with (c, k, t) free order, partition-major
        xs = np.ascontiguousarray(
            xT.reshape(KD, 128, NCH, CH).transpose(1, 2, 0, 3).reshape(128, -1)
        )
        cmask = np.full((PR, 1), float(half), np.float32)
        in_maps.append(
            {
                "x8": np.clip(xs, -240, 240).astype(f8),
                "xb": xs.astype(bf),
                "wf8": wf8,
                "wb": wb,
                "wlam": wlam,
                "wcw": wcw,
                "mcw": mcw,
                "cmask": cmask,
            }
        )
    return in_maps


def kernel(**inputs) -> np.ndarray:
    inputs = {k: np.asarray(v) for k, v in inputs.items()}
    if "nc" not in _CACHE:
        _CACHE["nc"] = build_program()
    nc = _CACHE["nc"]
    in_maps = _prep_inputs(**inputs)
    trace = bool(int(os.environ.get("CEPTA_TRACE", "0")))
    res = bass_utils.run_bass_kernel_spmd(
        nc,
        in_maps,
        core_ids=list(range(NCORES)),
        trace=trace,
        trace_cores=[0] if trace else None,
    )
    _CACHE["last_result"] = res
    out = np.empty((B, T, D), np.float32)
    for c in range(NCORES):
        b, half = c // 2, c % 2
        out[b, half * TL : (half + 1) * TL, :] = res.results[c]["h"].astype(
            np.float32
        )
    return out
